# revision 11
# baseline (speedup 1.0000x reference)
r"""Trainium2 Bass kernel for DeepRBFNetwork distances.

Math: distances[b, k] = || features[b] @ A[k].T + b[k] ||_2
  features: (4096, 512) f32, A: (100, 512, 512) f32, b: (100, 512) f32
  -> distances: (4096, 100) f32

Decomposition: with t = features @ A[k].T,
  S[b,k] = sum_e (t + b_k)^2 = sum_e t^2  +  f_b . (2 A_k^T b_k)  +  ||b_k||^2
           \__ Q: matmul+square __/   \__ affine: tiny matmul __/   \_ gB _/
  distances = sqrt(S)

Sharding: K padded 100->104, 13 classes per core across 8 NeuronCores; every
core sees the full batch. All operands are SBUF-resident (no streaming).

Device pipeline per core:
  - affine pre-phase: psum[128b,13k] = fT.T @ (2 A^T b) per batch tile,
    ACT Identity -> Saff (SBUF).
  - main: flat groups of 4 (bt,k) psum banks: matmuls (bf16 4-chunk accumulate,
    or fp8e4m3 DoubleRow 2x256-row accumulate with A pre-scaled by 2^12),
    one wide ACT Square over the 4 banks (descale via ACT's free affine
    scale), one DVE 3-D tensor_reduce -> Q columns.
  - per batch tile: S = Q + Saff + gB (DVE), ACT Sqrt, DMA out.

fp8 accuracy: output is dominated by the b=0.5 rows (distances ~11.31 with
~2e-4 relative spread); quantizing f, A to e4m3 perturbs distances by ~1e-5
relative. A must be pre-scaled by 2^12 because its ~1e-4 entries underflow
e4m3's 2^-9 minimum subnormal.
"""

import os
import sys
import types
import numpy as np
import ml_dtypes

import concourse.bacc as bacc
import concourse.bass as bass
import concourse.mybir as mybir
import concourse.tile as tile
from concourse.bass_utils import run_bass_kernel_spmd

B, K, D = 4096, 100, 512
NCORES = 8
KPAD = 104            # 8 * 13
KSH = KPAD // NCORES  # 13 classes per core
NBT = B // 128        # 32 batch tiles
NCH = D // 128        # 4 contraction chunks
G = 4                 # psum banks per epilogue group

BF16 = mybir.dt.bfloat16
FP8 = mybir.dt.float8e4
F32 = mybir.dt.float32
AF = mybir.ActivationFunctionType
ALU = mybir.AluOpType

A_SCALE_LOG2 = 12     # fp8: A pre-scaled by 2^12
C2_SCALE_LOG2 = 8     # fp8: c2 pre-scaled by 2^8

LAST_EXEC_TIME_NS = None
LAST_RESULTS = None

MODE = os.environ.get("BASS_KERNEL_MODE", "fp8")  # "fp8" | "bf16"


def build_nc(mode: str = MODE, n_bt: int = NBT):
    fp8 = mode == "fp8"
    mm_dt = FP8 if fp8 else BF16
    nc = bacc.Bacc(
        "TRN2", target_bir_lowering=False, debug=False, num_devices=NCORES
    )
    ftd = nc.dram_tensor("ftd", [128, 16384], mm_dt, kind="ExternalInput")
    atd = nc.dram_tensor("atd", [KSH, 128, NCH * D], mm_dt, kind="ExternalInput")
    c2d = nc.dram_tensor("c2d", [128, NCH * KSH], BF16, kind="ExternalInput")
    gBd = nc.dram_tensor("gBd", [128, KSH], F32, kind="ExternalInput")
    if fp8:
        # bf16 copy of fT for the affine matmul (accuracy: the f-quantization
        # error couples to the large b-bias through the affine term)
        ftbd = nc.dram_tensor("ftbd", [128, 16384], BF16, kind="ExternalInput")
    out = nc.dram_tensor("dist", [n_bt * 128, KSH], F32, kind="ExternalOutput")
    SQB = 4  # batch tiles per sqrt/output batch

    with tile.TileContext(nc) as tc:
        with (
            tc.tile_pool(name="const", bufs=1) as cpool,
            tc.tile_pool(name="gpsum", bufs=2, space="PSUM") as gpool,
            tc.tile_pool(name="sqp", bufs=2) as sqpool,
            tc.tile_pool(name="outp", bufs=3) as opool,
        ):
            if fp8:
                ft_t = cpool.tile([128, 2, 2, B], FP8, tag="ft")
            else:
                ft_t = cpool.tile([128, NCH * B], BF16, tag="ft")
            nc.sync.dma_start(ft_t[:], ftd[:])
            if fp8:
                at_t = cpool.tile([128, KSH, 2, 2, D], FP8, tag="at")
                for k in range(KSH):
                    nc.sync.dma_start(at_t[:, k], atd[k])
            else:
                at_t = cpool.tile([128, KSH * NCH * D], BF16, tag="at")
                for k in range(KSH):
                    nc.sync.dma_start(
                        at_t[:, k * NCH * D:(k + 1) * NCH * D], atd[k]
                    )
            c2_t = cpool.tile([128, NCH * KSH], BF16, tag="c2")
            nc.sync.dma_start(c2_t[:], c2d[:])
            gB_t = cpool.tile([128, KSH], F32, tag="gB")
            nc.sync.dma_start(gB_t[:], gBd[:])
            if fp8:
                ftb_t = cpool.tile([128, NCH * B], BF16, tag="ftb")
                nc.sync.dma_start(ftb_t[:], ftbd[:])
            else:
                ftb_t = ft_t

            qbig = cpool.tile([128, n_bt * KSH], F32, tag="qbig")

            def lhsb_slice(c, bt):
                # [128, 128] bf16 lhsT for contraction chunk c, batch tile bt
                return ftb_t[:, c * B + bt * 128: c * B + (bt + 1) * 128]

            sq_scale = 2.0 ** -A_SCALE_LOG2 if fp8 else 1.0
            flat = [(bt, k) for bt in range(n_bt) for k in range(KSH)]
            sgs = [flat[i:i + 2 * G] for i in range(0, len(flat), 2 * G)]
            done_upto = 0
            s4_tile = [None]

            def emit_assembly(bt):
                # affine matmul straight into a psum slot, consumed by DVE
                apg = gpool.tile([128, G, D], F32, tag="pg")
                aff = apg[:, 0, :KSH]
                for c in range(NCH):
                    nc.tensor.matmul(
                        aff,
                        lhsb_slice(c, bt),
                        c2_t[:, c * KSH:(c + 1) * KSH],
                        start=(c == 0),
                        stop=(c == NCH - 1),
                    )
                j = bt % SQB
                if j == 0:
                    s4_tile[0] = opool.tile([128, SQB, KSH], F32, tag="s4", name="s4")
                s4 = s4_tile[0]
                s_t = opool.tile([128, KSH], F32, tag="s")
                nc.vector.tensor_tensor(
                    s_t[:], aff, qbig[:, bt * KSH:(bt + 1) * KSH], op=ALU.add
                )
                nc.vector.tensor_tensor(s4[:, j, :], s_t[:], gB_t[:], op=ALU.add)
                if j == SQB - 1 or bt == n_bt - 1:
                    nn = j + 1
                    d4 = opool.tile([128, SQB, KSH], F32, tag="d4")
                    nc.scalar.activation(d4[:, :nn, :], s4[:, :nn, :], AF.Sqrt)
                    for jj in range(nn):
                        bx = bt - nn + 1 + jj
                        nc.sync.dma_start(
                            out[bx * 128:(bx + 1) * 128, :], d4[:, jj, :]
                        )

            for si, sg in enumerate(sgs):
                sq = sqpool.tile([128, 2 * G, D], BF16, tag="sq")
                for h in range(2):
                    half = sg[h * G:(h + 1) * G]
                    if not half:
                        continue
                    pg = gpool.tile([128, G, D], F32, tag="pg")
                    for j, (bt, k) in enumerate(half):
                        if fp8:
                            for pr in range(2):
                                nc.tensor.matmul(
                                    pg[:, j, :],
                                    ft_t[:, pr, :, bt * 128:(bt + 1) * 128],
                                    at_t[:, k, pr],
                                    start=(pr == 0),
                                    stop=(pr == 1),
                                    perf_mode=mybir.MatmulPerfMode.DoubleRow,
                                )
                        else:
                            for c in range(NCH):
                                nc.tensor.matmul(
                                    pg[:, j, :],
                                    lhsb_slice(c, bt),
                                    at_t[:, (k * NCH + c) * D:(k * NCH + c + 1) * D],
                                    start=(c == 0),
                                    stop=(c == NCH - 1),
                                )
                    nh = len(half)
                    nc.scalar.activation(
                        sq[:, h * G:h * G + nh, :], pg[:, :nh, :],
                        AF.Square, scale=sq_scale,
                    )
                ntot = len(sg)
                base = si * 2 * G
                nc.vector.tensor_reduce(
                    qbig[:, base:base + ntot], sq[:, :ntot, :],
                    axis=mybir.AxisListType.X, op=ALU.add,
                )
                # emit assembly for every bt fully reduced by this super-group
                new_done = min((base + ntot) // KSH, n_bt)
                for bt in range(done_upto, new_done):
                    emit_assembly(bt)
                done_upto = max(done_upto, new_done)
            for bt in range(done_upto, n_bt):
                emit_assembly(bt)
    nc.compile()
    return nc


def prep_inputs(features, A, b, mode: str = MODE):
    """Host-side layout prep: transpose + pad + cast, split into 8 shards."""
    fp8 = mode == "fp8"
    np8 = mybir.dt.np(FP8)
    bf = ml_dtypes.bfloat16

    fT = np.ascontiguousarray(features.T)                  # [512, 4096]
    ftb_host = np.ascontiguousarray(
        fT.reshape(NCH, 128, B).transpose(1, 0, 2).reshape(128, NCH * B)
    ).astype(bf)
    if fp8:
        # [128, pair, intl, B]: element (p, pr, i, b) = fT[(2pr+i)*128+p, b]
        ft_host = np.ascontiguousarray(
            fT.reshape(2, 2, 128, B).transpose(2, 0, 1, 3)
        ).astype(np8)
    else:
        ft_host = ftb_host

    Ap = np.zeros((KPAD, D, D), dtype=np.float32)
    Ap[:K] = A
    bp = np.zeros((KPAD, D), dtype=np.float32)
    bp[:K] = b
    c2 = 2.0 * np.einsum('ked,ke->kd', Ap, bp)             # [KPAD, 512]
    g = np.sum(bp * bp, axis=1)                            # [KPAD]

    in_maps = []
    for i in range(NCORES):
        sl = slice(i * KSH, (i + 1) * KSH)
        AT = Ap[sl].transpose(0, 2, 1)                     # [13, 512(d), 512(e)]
        if fp8:
            at_host = np.ascontiguousarray(
                (AT * 2.0 ** A_SCALE_LOG2)
                .reshape(KSH, 2, 2, 128, D).transpose(0, 3, 1, 2, 4)
                .reshape(KSH, 128, NCH * D)
            ).astype(np8)
        else:
            at_host = np.ascontiguousarray(
                AT.reshape(KSH, NCH, 128, D).transpose(0, 2, 1, 3)
                .reshape(KSH, 128, NCH * D)
            ).astype(bf)
        c2T = np.ascontiguousarray(c2[sl].T)               # [512, 13]
        c2_host = np.ascontiguousarray(
            c2T.reshape(NCH, 128, KSH).transpose(1, 0, 2).reshape(128, NCH * KSH)
        ).astype(bf)
        gB_host = np.ascontiguousarray(
            np.broadcast_to(g[sl][None, :], (128, KSH))
        ).astype(np.float32)
        im = {
            "ftd": ft_host.reshape(128, 16384),
            "atd": at_host,
            "c2d": c2_host,
            "gBd": gB_host,
        }
        if fp8:
            im["ftbd"] = ftb_host
        in_maps.append(im)
    return in_maps


def _install_ntff_hook():
    """Register the axon NTFF profile hook (missing antenv.axon_hooks shim)."""
    try:
        import antenv.axon_hooks  # noqa: F401
        return True
    except ImportError:
        pass
    try:
        sys.path.insert(0, "/root/.axon_site")
        from trn_agent_boot.trn_boot import _ntff_profile_via_ctypes
        hook = _ntff_profile_via_ctypes("/opt/axon/libaxon_pjrt.so")
        if hook is None:
            return False
        import antenv
        mod = types.ModuleType("antenv.axon_hooks")
        mod._hook = hook
        mod.get_axon_ntff_profile_hook = lambda: mod._hook
        mod.set_axon_ntff_profile_hook = lambda h: setattr(mod, "_hook", h)
        sys.modules["antenv.axon_hooks"] = mod
        antenv.axon_hooks = mod
        return True
    except Exception as e:  # pragma: no cover
        print(f"ntff hook install failed: {e}", file=sys.stderr)
        return False


def kernel(features: np.ndarray, A: np.ndarray, b: np.ndarray) -> np.ndarray:
    global LAST_EXEC_TIME_NS, LAST_RESULTS
    trace = bool(os.environ.get("BASS_KERNEL_TRACE"))
    kwargs = {}
    if trace:
        if _install_ntff_hook():
            import concourse.bass_utils as bu
            bu.upload_artifacts = lambda tmpdir: f"local:{tmpdir}"
            tmpdir = os.environ.get("BASS_KERNEL_TRACE_DIR") or None
            if tmpdir:
                import glob as _glob
                for f in _glob.glob(os.path.join(tmpdir, "*")):
                    try:
                        os.remove(f)
                    except OSError:
                        pass
            kwargs = dict(trace=True, tmpdir=tmpdir)
        else:
            print("trace requested but NTFF hook unavailable", file=sys.stderr)

    nc = build_nc(MODE, NBT)
    in_maps = prep_inputs(
        np.asarray(features, dtype=np.float32),
        np.asarray(A, dtype=np.float32),
        np.asarray(b, dtype=np.float32),
        MODE,
    )
    res = run_bass_kernel_spmd(nc, in_maps, list(range(NCORES)), **kwargs)
    LAST_RESULTS = res
    LAST_EXEC_TIME_NS = res.exec_time_ns
    full = np.concatenate([res.results[i]["dist"] for i in range(NCORES)], axis=1)
    return np.ascontiguousarray(full[:, :K]).astype(np.float32)


# revision 13
# speedup vs baseline: 1.7664x; 1.7664x over previous
r"""Trainium2 Bass kernel for DeepRBFNetwork distances.

Math: distances[b, k] = || features[b] @ A[k].T + b[k] ||_2
  features: (4096, 512) f32, A: (100, 512, 512) f32, b: (100, 512) f32
  -> distances: (4096, 100) f32

Decomposition: with t = features @ A[k].T,
  S[b,k] = sum_e (t + b_k)^2 = sum_e t^2  +  f_b . (2 A_k^T b_k)  +  ||b_k||^2
           \__ Q: matmul+square __/   \__ affine: tiny matmul __/   \_ gB _/
  distances = sqrt(S)

Sharding: K padded 100->104, 13 classes per core across 8 NeuronCores; every
core sees the full batch. All operands are SBUF-resident (no streaming).

Device pipeline per core:
  - affine pre-phase: psum[128b,13k] = fT.T @ (2 A^T b) per batch tile,
    ACT Identity -> Saff (SBUF).
  - main: flat groups of 4 (bt,k) psum banks: matmuls (bf16 4-chunk accumulate,
    or fp8e4m3 DoubleRow 2x256-row accumulate with A pre-scaled by 2^12),
    one wide ACT Square over the 4 banks (descale via ACT's free affine
    scale), one DVE 3-D tensor_reduce -> Q columns.
  - per batch tile: S = Q + Saff + gB (DVE), ACT Sqrt, DMA out.

fp8 accuracy: output is dominated by the b=0.5 rows (distances ~11.31 with
~2e-4 relative spread); quantizing f, A to e4m3 perturbs distances by ~1e-5
relative. A must be pre-scaled by 2^12 because its ~1e-4 entries underflow
e4m3's 2^-9 minimum subnormal.
"""

import os
import sys
import types
import numpy as np
import ml_dtypes

import concourse.bacc as bacc
import concourse.bass as bass
import concourse.mybir as mybir
import concourse.tile as tile
from concourse.bass_utils import run_bass_kernel_spmd

B, K, D = 4096, 100, 512
NCORES = 8
KPAD = 104            # 8 * 13
KSH = KPAD // NCORES  # 13 classes per core
NBT = B // 128        # 32 batch tiles
NCH = D // 128        # 4 contraction chunks
G = 4                 # psum banks per epilogue group

BF16 = mybir.dt.bfloat16
FP8 = mybir.dt.float8e4
F32 = mybir.dt.float32
AF = mybir.ActivationFunctionType
ALU = mybir.AluOpType

A_SCALE_LOG2 = 12     # fp8: A pre-scaled by 2^12
C2_SCALE_LOG2 = 8     # fp8: c2 pre-scaled by 2^8

LAST_EXEC_TIME_NS = None
LAST_RESULTS = None

MODE = os.environ.get("BASS_KERNEL_MODE", "fp8")  # "fp8" | "bf16"


def build_nc(mode: str = MODE, n_bt: int = NBT):
    fp8 = mode == "fp8"
    mm_dt = FP8 if fp8 else BF16
    nc = bacc.Bacc(
        "TRN2", target_bir_lowering=False, debug=False, num_devices=NCORES
    )
    ftd = nc.dram_tensor("ftd", [128, 16384], mm_dt, kind="ExternalInput")
    atd = nc.dram_tensor("atd", [KSH, 128, NCH * D], mm_dt, kind="ExternalInput")
    c2d = nc.dram_tensor("c2d", [128, NCH * KSH], BF16, kind="ExternalInput")
    gBd = nc.dram_tensor("gBd", [128, KSH], F32, kind="ExternalInput")
    if fp8:
        # bf16 copy of fT for the affine matmul (accuracy: the f-quantization
        # error couples to the large b-bias through the affine term)
        ftbd = nc.dram_tensor("ftbd", [128, 16384], BF16, kind="ExternalInput")
    out = nc.dram_tensor("dist", [n_bt * 128, KSH], F32, kind="ExternalOutput")
    SQB = 4  # batch tiles per sqrt/output batch

    with tile.TileContext(nc) as tc:
        with (
            tc.tile_pool(name="const", bufs=1) as cpool,
            tc.tile_pool(name="gpsum", bufs=2, space="PSUM") as gpool,
            tc.tile_pool(name="sqp", bufs=3) as sqpool,
            tc.tile_pool(name="outp", bufs=3) as opool,
        ):
            if fp8:
                ft_t = cpool.tile([128, 2, 2, B], FP8, tag="ft")
            else:
                ft_t = cpool.tile([128, NCH * B], BF16, tag="ft")
            nc.sync.dma_start(ft_t[:], ftd[:])
            if fp8:
                at_t = cpool.tile([128, KSH, 2, 2, D], FP8, tag="at")
                for k in range(KSH):
                    nc.sync.dma_start(at_t[:, k], atd[k])
            else:
                at_t = cpool.tile([128, KSH * NCH * D], BF16, tag="at")
                for k in range(KSH):
                    nc.sync.dma_start(
                        at_t[:, k * NCH * D:(k + 1) * NCH * D], atd[k]
                    )
            c2_t = cpool.tile([128, NCH * KSH], BF16, tag="c2")
            nc.sync.dma_start(c2_t[:], c2d[:])
            gB_t = cpool.tile([128, KSH], F32, tag="gB")
            nc.sync.dma_start(gB_t[:], gBd[:])
            if fp8:
                ftb_t = cpool.tile([128, NCH * B], BF16, tag="ftb")
                nc.sync.dma_start(ftb_t[:], ftbd[:])
            else:
                ftb_t = ft_t

            qbig = cpool.tile([128, n_bt * KSH], F32, tag="qbig")
            saff = cpool.tile([128, n_bt * KSH], F32, tag="saff")

            def lhsb_slice(c, bt):
                # [128, 128] bf16 lhsT for contraction chunk c, batch tile bt
                return ftb_t[:, c * B + bt * 128: c * B + (bt + 1) * 128]

            # ---- affine pre-phase: Saff[:, bt*13+k] = f . c2 + overlaps the
            # at-shard DMA (only needs ft + c2) ----
            for bt in range(n_bt):
                apg = gpool.tile([128, G, D], F32, tag="pg")
                aff = apg[:, 0, :KSH]
                for c in range(NCH):
                    nc.tensor.matmul(
                        aff,
                        lhsb_slice(c, bt),
                        c2_t[:, c * KSH:(c + 1) * KSH],
                        start=(c == 0),
                        stop=(c == NCH - 1),
                    )
                nc.scalar.activation(
                    saff[:, bt * KSH:(bt + 1) * KSH], aff, AF.Identity
                )

            sq_scale = 2.0 ** -A_SCALE_LOG2 if fp8 else 1.0
            flat = [(bt, k) for bt in range(n_bt) for k in range(KSH)]
            sgs = [flat[i:i + 2 * G] for i in range(0, len(flat), 2 * G)]
            done_upto = 0
            s4_tile = [None]

            def emit_assembly(bt):
                j = bt % SQB
                if j == 0:
                    s4_tile[0] = opool.tile([128, SQB, KSH], F32, tag="s4", name="s4")
                s4 = s4_tile[0]
                s_t = opool.tile([128, KSH], F32, tag="s")
                nc.vector.tensor_tensor(
                    s_t[:], saff[:, bt * KSH:(bt + 1) * KSH],
                    qbig[:, bt * KSH:(bt + 1) * KSH], op=ALU.add
                )
                nc.vector.tensor_tensor(s4[:, j, :], s_t[:], gB_t[:], op=ALU.add)
                if j == SQB - 1 or bt == n_bt - 1:
                    nn = j + 1
                    d4 = opool.tile([128, SQB, KSH], F32, tag="d4")
                    nc.scalar.activation(d4[:, :nn, :], s4[:, :nn, :], AF.Sqrt)
                    for jj in range(nn):
                        bx = bt - nn + 1 + jj
                        nc.sync.dma_start(
                            out[bx * 128:(bx + 1) * 128, :], d4[:, jj, :]
                        )

            for si, sg in enumerate(sgs):
                sq = sqpool.tile([128, 2 * G, D], BF16, tag="sq")
                for h in range(2):
                    half = sg[h * G:(h + 1) * G]
                    if not half:
                        continue
                    pg = gpool.tile([128, G, D], F32, tag="pg")
                    for j, (bt, k) in enumerate(half):
                        if fp8:
                            for pr in range(2):
                                nc.tensor.matmul(
                                    pg[:, j, :],
                                    ft_t[:, pr, :, bt * 128:(bt + 1) * 128],
                                    at_t[:, k, pr],
                                    start=(pr == 0),
                                    stop=(pr == 1),
                                    perf_mode=mybir.MatmulPerfMode.DoubleRow,
                                )
                        else:
                            for c in range(NCH):
                                nc.tensor.matmul(
                                    pg[:, j, :],
                                    lhsb_slice(c, bt),
                                    at_t[:, (k * NCH + c) * D:(k * NCH + c + 1) * D],
                                    start=(c == 0),
                                    stop=(c == NCH - 1),
                                )
                    nh = len(half)
                    nc.scalar.activation(
                        sq[:, h * G:h * G + nh, :], pg[:, :nh, :],
                        AF.Square, scale=sq_scale,
                    )
                ntot = len(sg)
                base = si * 2 * G
                nc.vector.tensor_reduce(
                    qbig[:, base:base + ntot], sq[:, :ntot, :],
                    axis=mybir.AxisListType.X, op=ALU.add,
                )
                # emit assembly for every bt fully reduced by this super-group
                new_done = min((base + ntot) // KSH, n_bt)
                for bt in range(done_upto, new_done):
                    emit_assembly(bt)
                done_upto = max(done_upto, new_done)
            for bt in range(done_upto, n_bt):
                emit_assembly(bt)
    nc.compile()
    return nc


def prep_inputs(features, A, b, mode: str = MODE):
    """Host-side layout prep: transpose + pad + cast, split into 8 shards."""
    fp8 = mode == "fp8"
    np8 = mybir.dt.np(FP8)
    bf = ml_dtypes.bfloat16

    fT = np.ascontiguousarray(features.T)                  # [512, 4096]
    ftb_host = np.ascontiguousarray(
        fT.reshape(NCH, 128, B).transpose(1, 0, 2).reshape(128, NCH * B)
    ).astype(bf)
    if fp8:
        # [128, pair, intl, B]: element (p, pr, i, b) = fT[(2pr+i)*128+p, b]
        ft_host = np.ascontiguousarray(
            fT.reshape(2, 2, 128, B).transpose(2, 0, 1, 3)
        ).astype(np8)
    else:
        ft_host = ftb_host

    Ap = np.zeros((KPAD, D, D), dtype=np.float32)
    Ap[:K] = A
    bp = np.zeros((KPAD, D), dtype=np.float32)
    bp[:K] = b
    c2 = 2.0 * np.einsum('ked,ke->kd', Ap, bp)             # [KPAD, 512]
    g = np.sum(bp * bp, axis=1)                            # [KPAD]

    in_maps = []
    for i in range(NCORES):
        sl = slice(i * KSH, (i + 1) * KSH)
        AT = Ap[sl].transpose(0, 2, 1)                     # [13, 512(d), 512(e)]
        if fp8:
            at_host = np.ascontiguousarray(
                (AT * 2.0 ** A_SCALE_LOG2)
                .reshape(KSH, 2, 2, 128, D).transpose(0, 3, 1, 2, 4)
                .reshape(KSH, 128, NCH * D)
            ).astype(np8)
        else:
            at_host = np.ascontiguousarray(
                AT.reshape(KSH, NCH, 128, D).transpose(0, 2, 1, 3)
                .reshape(KSH, 128, NCH * D)
            ).astype(bf)
        c2T = np.ascontiguousarray(c2[sl].T)               # [512, 13]
        c2_host = np.ascontiguousarray(
            c2T.reshape(NCH, 128, KSH).transpose(1, 0, 2).reshape(128, NCH * KSH)
        ).astype(bf)
        gB_host = np.ascontiguousarray(
            np.broadcast_to(g[sl][None, :], (128, KSH))
        ).astype(np.float32)
        im = {
            "ftd": ft_host.reshape(128, 16384),
            "atd": at_host,
            "c2d": c2_host,
            "gBd": gB_host,
        }
        if fp8:
            im["ftbd"] = ftb_host
        in_maps.append(im)
    return in_maps


def _install_ntff_hook():
    """Register the axon NTFF profile hook (missing antenv.axon_hooks shim)."""
    try:
        import antenv.axon_hooks  # noqa: F401
        return True
    except ImportError:
        pass
    try:
        sys.path.insert(0, "/root/.axon_site")
        from trn_agent_boot.trn_boot import _ntff_profile_via_ctypes
        hook = _ntff_profile_via_ctypes("/opt/axon/libaxon_pjrt.so")
        if hook is None:
            return False
        import antenv
        mod = types.ModuleType("antenv.axon_hooks")
        mod._hook = hook
        mod.get_axon_ntff_profile_hook = lambda: mod._hook
        mod.set_axon_ntff_profile_hook = lambda h: setattr(mod, "_hook", h)
        sys.modules["antenv.axon_hooks"] = mod
        antenv.axon_hooks = mod
        return True
    except Exception as e:  # pragma: no cover
        print(f"ntff hook install failed: {e}", file=sys.stderr)
        return False


def kernel(features: np.ndarray, A: np.ndarray, b: np.ndarray) -> np.ndarray:
    global LAST_EXEC_TIME_NS, LAST_RESULTS
    trace = bool(os.environ.get("BASS_KERNEL_TRACE"))
    kwargs = {}
    if trace:
        if _install_ntff_hook():
            import concourse.bass_utils as bu
            bu.upload_artifacts = lambda tmpdir: f"local:{tmpdir}"
            tmpdir = os.environ.get("BASS_KERNEL_TRACE_DIR") or None
            if tmpdir:
                import glob as _glob
                for f in _glob.glob(os.path.join(tmpdir, "*")):
                    try:
                        os.remove(f)
                    except OSError:
                        pass
            kwargs = dict(trace=True, tmpdir=tmpdir)
        else:
            print("trace requested but NTFF hook unavailable", file=sys.stderr)

    nc = build_nc(MODE, NBT)
    in_maps = prep_inputs(
        np.asarray(features, dtype=np.float32),
        np.asarray(A, dtype=np.float32),
        np.asarray(b, dtype=np.float32),
        MODE,
    )
    res = run_bass_kernel_spmd(nc, in_maps, list(range(NCORES)), **kwargs)
    LAST_RESULTS = res
    LAST_EXEC_TIME_NS = res.exec_time_ns
    full = np.concatenate([res.results[i]["dist"] for i in range(NCORES)], axis=1)
    return np.ascontiguousarray(full[:, :K]).astype(np.float32)


# revision 16
# speedup vs baseline: 1.8082x; 1.0237x over previous
r"""Trainium2 Bass kernel for DeepRBFNetwork distances.

Math: distances[b, k] = || features[b] @ A[k].T + b[k] ||_2
  features: (4096, 512) f32, A: (100, 512, 512) f32, b: (100, 512) f32
  -> distances: (4096, 100) f32

Decomposition: with t = features @ A[k].T,
  S[b,k] = sum_e (t + b_k)^2 = sum_e t^2  +  f_b . (2 A_k^T b_k)  +  ||b_k||^2
           \__ Q: matmul+square __/   \__ affine: tiny matmul __/   \_ gB _/
  distances = sqrt(S)

Sharding: K padded 100->104, 13 classes per core across 8 NeuronCores; every
core sees the full batch. All operands are SBUF-resident (no streaming).

Device pipeline per core:
  - affine pre-phase: psum[128b,13k] = fT.T @ (2 A^T b) per batch tile,
    ACT Identity -> Saff (SBUF).
  - main: flat groups of 4 (bt,k) psum banks: matmuls (bf16 4-chunk accumulate,
    or fp8e4m3 DoubleRow 2x256-row accumulate with A pre-scaled by 2^12),
    one wide ACT Square over the 4 banks (descale via ACT's free affine
    scale), one DVE 3-D tensor_reduce -> Q columns.
  - per batch tile: S = Q + Saff + gB (DVE), ACT Sqrt, DMA out.

fp8 accuracy: output is dominated by the b=0.5 rows (distances ~11.31 with
~2e-4 relative spread); quantizing f, A to e4m3 perturbs distances by ~1e-5
relative. A must be pre-scaled by 2^12 because its ~1e-4 entries underflow
e4m3's 2^-9 minimum subnormal.
"""

import os
import sys
import types
import numpy as np
import ml_dtypes

import concourse.bacc as bacc
import concourse.bass as bass
import concourse.mybir as mybir
import concourse.tile as tile
from concourse.bass_utils import run_bass_kernel_spmd

B, K, D = 4096, 100, 512
NCORES = 8
KPAD = 104            # 8 * 13
KSH = KPAD // NCORES  # 13 classes per core
NBT = B // 128        # 32 batch tiles
NCH = D // 128        # 4 contraction chunks
G = 4                 # psum banks per epilogue group

BF16 = mybir.dt.bfloat16
FP8 = mybir.dt.float8e4
F32 = mybir.dt.float32
AF = mybir.ActivationFunctionType
ALU = mybir.AluOpType

A_SCALE_LOG2 = 12     # fp8: A pre-scaled by 2^12
C2_SCALE_LOG2 = 8     # fp8: c2 pre-scaled by 2^8

LAST_EXEC_TIME_NS = None
LAST_RESULTS = None

MODE = os.environ.get("BASS_KERNEL_MODE", "fp8")  # "fp8" | "bf16"


def build_nc(mode: str = MODE, n_bt: int = NBT):
    fp8 = mode == "fp8"
    mm_dt = FP8 if fp8 else BF16
    nc = bacc.Bacc(
        "TRN2", target_bir_lowering=False, debug=False, num_devices=NCORES
    )
    ftd = nc.dram_tensor("ftd", [128, 16384], mm_dt, kind="ExternalInput")
    atd = nc.dram_tensor("atd", [KSH, 128, NCH * D], mm_dt, kind="ExternalInput")
    c2d = nc.dram_tensor("c2d", [128, NCH * KSH], BF16, kind="ExternalInput")
    g2d = nc.dram_tensor("g2d", [2, KSH], BF16, kind="ExternalInput")
    if fp8:
        # bf16 copy of fT for the affine matmul (accuracy: the f-quantization
        # error couples to the large b-bias through the affine term)
        ftbd = nc.dram_tensor("ftbd", [128, 16384], BF16, kind="ExternalInput")
    out = nc.dram_tensor("dist", [n_bt * 128, KSH], F32, kind="ExternalOutput")
    SQB = 4   # batch tiles per sqrt/output batch
    SGW = 16  # (bt, k) pairs per super-group (one DVE reduce)
    LAG = 13  # super-groups between main loop and trailing affine emission

    with tile.TileContext(nc) as tc:
        with (
            tc.tile_pool(name="const", bufs=1) as cpool,
            tc.tile_pool(name="gpsum", bufs=2, space="PSUM") as gpool,
            tc.tile_pool(name="sqp", bufs=3) as sqpool,
            tc.tile_pool(name="outp", bufs=3) as opool,
        ):
            # DMA order: main-loop operands first (ft, early at shards), then
            # the affine operands (c2, g2, ftb) which are needed ~LAG
            # super-groups in.
            if fp8:
                ft_t = cpool.tile([128, 2, 2, B], FP8, tag="ft")
            else:
                ft_t = cpool.tile([128, NCH * B], BF16, tag="ft")
            nc.sync.dma_start(ft_t[:], ftd[:])
            if fp8:
                at_t = cpool.tile([128, KSH, 2, 2, D], FP8, tag="at")
                for k in range(KSH):
                    nc.sync.dma_start(at_t[:, k], atd[k])
            else:
                at_t = cpool.tile([128, KSH * NCH * D], BF16, tag="at")
                for k in range(KSH):
                    nc.sync.dma_start(
                        at_t[:, k * NCH * D:(k + 1) * NCH * D], atd[k]
                    )
            c2_t = cpool.tile([128, NCH * KSH], BF16, tag="c2")
            nc.sync.dma_start(c2_t[:], c2d[:])
            g2_t = cpool.tile([2, KSH], BF16, tag="g2")
            nc.sync.dma_start(g2_t[:], g2d[:])
            if fp8:
                ftb_t = cpool.tile([128, NCH * B], BF16, tag="ftb")
                nc.sync.dma_start(ftb_t[:], ftbd[:])
            else:
                ftb_t = ft_t
            ones2 = cpool.tile([2, B], BF16, tag="ones2")
            nc.gpsimd.memset(ones2[:], 1.0)

            qbig = cpool.tile([128, n_bt * KSH], F32, tag="qbig")
            saff = cpool.tile([128, n_bt * KSH], F32, tag="saff")

            def lhsb_slice(c, bt):
                # [128, 128] bf16 lhsT for contraction chunk c, batch tile bt
                return ftb_t[:, c * B + bt * 128: c * B + (bt + 1) * 128]

            def emit_affine(bt):
                # Saff[:, bt] = f . c2 + g (g via a 2-row bf16-exact hi/lo
                # contraction against a ones lhsT)
                apg = gpool.tile([128, G, D], F32, tag="pg")
                aff = apg[:, 0, :KSH]
                for c in range(NCH):
                    nc.tensor.matmul(
                        aff,
                        lhsb_slice(c, bt),
                        c2_t[:, c * KSH:(c + 1) * KSH],
                        start=(c == 0),
                        stop=False,
                    )
                nc.tensor.matmul(
                    aff, ones2[:, bt * 128:(bt + 1) * 128], g2_t[:],
                    start=False, stop=True,
                )
                nc.scalar.activation(
                    saff[:, bt * KSH:(bt + 1) * KSH], aff, AF.Identity
                )

            sq_scale = 2.0 ** -A_SCALE_LOG2 if fp8 else 1.0
            flat = [(bt, k) for bt in range(n_bt) for k in range(KSH)]
            sgs = [flat[i:i + SGW] for i in range(0, len(flat), SGW)]
            done_upto = 0
            aff_done = 0
            s4_tile = [None]

            def emit_assembly(bt):
                j = bt % SQB
                if j == 0:
                    s4_tile[0] = opool.tile([128, SQB, KSH], F32, tag="s4", name="s4")
                s4 = s4_tile[0]
                nc.vector.tensor_tensor(
                    s4[:, j, :], saff[:, bt * KSH:(bt + 1) * KSH],
                    qbig[:, bt * KSH:(bt + 1) * KSH], op=ALU.add
                )
                if j == SQB - 1 or bt == n_bt - 1:
                    nn = j + 1
                    d4 = opool.tile([128, SQB, KSH], F32, tag="d4")
                    nc.scalar.activation(d4[:, :nn, :], s4[:, :nn, :], AF.Sqrt)
                    for jj in range(nn):
                        bx = bt - nn + 1 + jj
                        nc.sync.dma_start(
                            out[bx * 128:(bx + 1) * 128, :], d4[:, jj, :]
                        )

            for si, sg in enumerate(sgs):
                sq = sqpool.tile([128, SGW, D], BF16, tag="sq")
                for h in range((len(sg) + G - 1) // G):
                    half = sg[h * G:(h + 1) * G]
                    pg = gpool.tile([128, G, D], F32, tag="pg")
                    for j, (bt, k) in enumerate(half):
                        if fp8:
                            for pr in range(2):
                                nc.tensor.matmul(
                                    pg[:, j, :],
                                    ft_t[:, pr, :, bt * 128:(bt + 1) * 128],
                                    at_t[:, k, pr],
                                    start=(pr == 0),
                                    stop=(pr == 1),
                                    perf_mode=mybir.MatmulPerfMode.DoubleRow,
                                )
                        else:
                            for c in range(NCH):
                                nc.tensor.matmul(
                                    pg[:, j, :],
                                    lhsb_slice(c, bt),
                                    at_t[:, (k * NCH + c) * D:(k * NCH + c + 1) * D],
                                    start=(c == 0),
                                    stop=(c == NCH - 1),
                                )
                    nh = len(half)
                    nc.scalar.activation(
                        sq[:, h * G:h * G + nh, :], pg[:, :nh, :],
                        AF.Square, scale=sq_scale,
                    )
                ntot = len(sg)
                base = si * SGW
                nc.vector.tensor_reduce(
                    qbig[:, base:base + ntot], sq[:, :ntot, :],
                    axis=mybir.AxisListType.X, op=ALU.add,
                )
                # trailing affine emission (operands arrive after the main
                # tensors; LAG keeps the PE FIFO from stalling on their DMA)
                if si >= LAG and aff_done < n_bt:
                    emit_affine(aff_done)
                    aff_done += 1
                # emit assembly for every bt fully reduced and affine-ready
                new_done = min((base + ntot) // KSH, n_bt, aff_done)
                for bt in range(done_upto, new_done):
                    emit_assembly(bt)
                done_upto = max(done_upto, new_done)
            while aff_done < n_bt:
                emit_affine(aff_done)
                aff_done += 1
            for bt in range(done_upto, n_bt):
                emit_assembly(bt)
    nc.compile()
    return nc


def prep_inputs(features, A, b, mode: str = MODE):
    """Host-side layout prep: transpose + pad + cast, split into 8 shards."""
    fp8 = mode == "fp8"
    np8 = mybir.dt.np(FP8)
    bf = ml_dtypes.bfloat16

    fT = np.ascontiguousarray(features.T)                  # [512, 4096]
    ftb_host = np.ascontiguousarray(
        fT.reshape(NCH, 128, B).transpose(1, 0, 2).reshape(128, NCH * B)
    ).astype(bf)
    if fp8:
        # [128, pair, intl, B]: element (p, pr, i, b) = fT[(2pr+i)*128+p, b]
        ft_host = np.ascontiguousarray(
            fT.reshape(2, 2, 128, B).transpose(2, 0, 1, 3)
        ).astype(np8)
    else:
        ft_host = ftb_host

    Ap = np.zeros((KPAD, D, D), dtype=np.float32)
    Ap[:K] = A
    bp = np.zeros((KPAD, D), dtype=np.float32)
    bp[:K] = b
    c2 = 2.0 * np.einsum('ked,ke->kd', Ap, bp)             # [KPAD, 512]
    g = np.sum(bp * bp, axis=1)                            # [KPAD]

    in_maps = []
    for i in range(NCORES):
        sl = slice(i * KSH, (i + 1) * KSH)
        AT = Ap[sl].transpose(0, 2, 1)                     # [13, 512(d), 512(e)]
        if fp8:
            at_host = np.ascontiguousarray(
                (AT * 2.0 ** A_SCALE_LOG2)
                .reshape(KSH, 2, 2, 128, D).transpose(0, 3, 1, 2, 4)
                .reshape(KSH, 128, NCH * D)
            ).astype(np8)
        else:
            at_host = np.ascontiguousarray(
                AT.reshape(KSH, NCH, 128, D).transpose(0, 2, 1, 3)
                .reshape(KSH, 128, NCH * D)
            ).astype(bf)
        c2T = np.ascontiguousarray(c2[sl].T)               # [512, 13]
        c2_host = np.ascontiguousarray(
            c2T.reshape(NCH, 128, KSH).transpose(1, 0, 2).reshape(128, NCH * KSH)
        ).astype(bf)
        g_hi = g[sl].astype(bf).astype(np.float32)
        g_lo = (g[sl] - g_hi).astype(bf)
        g2_host = np.ascontiguousarray(
            np.stack([g_hi.astype(bf), g_lo], axis=0)
        )
        im = {
            "ftd": ft_host.reshape(128, 16384),
            "atd": at_host,
            "c2d": c2_host,
            "g2d": g2_host,
        }
        if fp8:
            im["ftbd"] = ftb_host
        in_maps.append(im)
    return in_maps


def _install_ntff_hook():
    """Register the axon NTFF profile hook (missing antenv.axon_hooks shim)."""
    try:
        import antenv.axon_hooks  # noqa: F401
        return True
    except ImportError:
        pass
    try:
        sys.path.insert(0, "/root/.axon_site")
        from trn_agent_boot.trn_boot import _ntff_profile_via_ctypes
        hook = _ntff_profile_via_ctypes("/opt/axon/libaxon_pjrt.so")
        if hook is None:
            return False
        import antenv
        mod = types.ModuleType("antenv.axon_hooks")
        mod._hook = hook
        mod.get_axon_ntff_profile_hook = lambda: mod._hook
        mod.set_axon_ntff_profile_hook = lambda h: setattr(mod, "_hook", h)
        sys.modules["antenv.axon_hooks"] = mod
        antenv.axon_hooks = mod
        return True
    except Exception as e:  # pragma: no cover
        print(f"ntff hook install failed: {e}", file=sys.stderr)
        return False


def kernel(features: np.ndarray, A: np.ndarray, b: np.ndarray) -> np.ndarray:
    global LAST_EXEC_TIME_NS, LAST_RESULTS
    trace = bool(os.environ.get("BASS_KERNEL_TRACE"))
    kwargs = {}
    if trace:
        if _install_ntff_hook():
            import concourse.bass_utils as bu
            bu.upload_artifacts = lambda tmpdir: f"local:{tmpdir}"
            tmpdir = os.environ.get("BASS_KERNEL_TRACE_DIR") or None
            if tmpdir:
                import glob as _glob
                for f in _glob.glob(os.path.join(tmpdir, "*")):
                    try:
                        os.remove(f)
                    except OSError:
                        pass
            kwargs = dict(trace=True, tmpdir=tmpdir)
        else:
            print("trace requested but NTFF hook unavailable", file=sys.stderr)

    nc = build_nc(MODE, NBT)
    in_maps = prep_inputs(
        np.asarray(features, dtype=np.float32),
        np.asarray(A, dtype=np.float32),
        np.asarray(b, dtype=np.float32),
        MODE,
    )
    res = run_bass_kernel_spmd(nc, in_maps, list(range(NCORES)), **kwargs)
    LAST_RESULTS = res
    LAST_EXEC_TIME_NS = res.exec_time_ns
    full = np.concatenate([res.results[i]["dist"] for i in range(NCORES)], axis=1)
    return np.ascontiguousarray(full[:, :K]).astype(np.float32)
